# revision 18
# baseline (speedup 1.0000x reference)
"""Channel-attention (CAM) Trainium2 kernel — fp8 DoubleRow mm2 version.

Reference computation (per batch b of 16):
    q   = x[b].reshape(C, HW)                  # C=512, HW=4096
    sim = q @ q.T                              # [C, C], symmetric
    attn = softmax(max(sim) - sim, axis=-1)    # == exp(min_r - sim) / Z_r
    out[b] = gamma * attn @ q + x[b]

Sharding: data-parallel over batch across 8 NeuronCores (2 batches/core).
kernel() takes full inputs, shards internally, returns the full output.

Per-core kernel design (v3):
  - ALL loads (both batches) are emitted up front on the Sync HWDGE queue;
    stores are emitted later on the same queue so store bytes can never
    delay load bytes. x lands directly in persistent qr f32 tiles.
  - sim (mm1) in f32r: winner-take-all softmax needs sim errors << the
    typical min-gap; f32r gives ~4e-4 end-to-end rel err. qT tiles are
    PE-transposed from qr (bf16 identity: 1 cycle/row) and rounded to
    f32r by the PSUM->SBUF copyback cast (BIR verifier requirement).
  - mm2 in fp8e4 with DoubleRow perf mode (2 k-rows/cycle, verified
    213ns per 512-col call on HW): p = exp(min-sim) is cast to fp8
    UNSCALED so the softmax winner is exactly 1.0 in fp8; q8 = fp8(q) is
    produced by gpsimd software-DGE cast-DMAs (SBUF->SBUF, zero vector
    engine cost). gamma/Z is applied post-matmul fused with the +x
    residual: chunks alternate DVE scalar_tensor_tensor (one pass) and
    ACT scale + Pool add (two ops) for engine balance. Accuracy ~1.1e-2
    rel L2, dominated by fp8 quantization of q weighted by gamma=0.435.
  - DoubleRow layout: q8 [128, 4(kd), 4096], p8T [128, 4(kd), 512(c)];
    each mm2 call contracts a kd pair: lhsT [128,2,128], rhs [128,2,512].
  - Schedule: batch-0's mm2 chunks interleave between batch-1's
    transpose/mm1 groups as PE filler, with a 2-chunk reserve to cover
    batch-1's softmax latency (HAM punishes PE gaps with ~3us of
    half-rate clock ramp).
  - 8 dummy identity matmuls at t=0 pre-warm the PE clock gate while the
    first loads are in flight.
"""
import sys

if "/opt/trn_rl_repo" not in sys.path:
    sys.path.insert(0, "/opt/trn_rl_repo")

import numpy as np

B, C, H, W = 16, 512, 64, 64
HW = H * W
NCORES = 8
NB = B // NCORES          # batches per core
P = 128
CB = C // P               # 4 channel blocks
KN = HW // P              # 32 contraction chunks for sim
NJ = HW // 512            # 8 output column chunks

_BUILD_CACHE = {}


def build_bass():
    import concourse.bacc as bacc
    import concourse.tile as tile
    from concourse import mybir
    from concourse.masks import make_identity

    f32 = mybir.dt.float32
    f32r = mybir.dt.float32r
    bf16 = mybir.dt.bfloat16
    fp8 = mybir.dt.float8e4
    AX = mybir.AxisListType
    ALU = mybir.AluOpType
    ACTF = mybir.ActivationFunctionType
    DR = mybir.MatmulPerfMode.DoubleRow

    nc = bacc.Bacc()
    x_ext = nc.declare_dram_parameter("x", [NB, C, HW], f32r, isOutput=False)
    g_ext = nc.declare_dram_parameter("gamma", [1], f32, isOutput=False)
    o_ext = nc.declare_dram_parameter("out", [NB, C, HW], f32, isOutput=True)

    _flip = [0]

    with tile.TileContext(nc) as tc:
        with (
            tc.tile_pool(name="const", bufs=1) as const,
            tc.tile_pool(name="qr", bufs=2 * CB) as qrp,
            tc.tile_pool(name="q8", bufs=2) as q8p,
            tc.tile_pool(name="qt", bufs=6) as qtp,
            tc.tile_pool(name="pp", bufs=5) as pp,
            tc.tile_pool(name="p8t", bufs=2) as p8tp,
            tc.tile_pool(name="osb", bufs=3) as osb,
            tc.tile_pool(name="tri", bufs=2) as trip,
            tc.tile_pool(name="vec", bufs=32) as vec,
            tc.tile_pool(name="psA", bufs=2, space="PSUM") as psA,
            tc.tile_pool(name="psim", bufs=4, space="PSUM") as psimp,
            tc.tile_pool(name="pfeat", bufs=2, space="PSUM") as pfeat,
        ):
            _cb = [0]

            def copyback(dst, src):
                # qt copybacks mostly on ACT; DVE owns the mm2 out-units
                if _cb[0] % 4 == 3:
                    nc.vector.tensor_copy(dst, src)
                else:
                    nc.scalar.copy(dst, src)
                _cb[0] += 1

            # x is declared f32r in DRAM (byte-identical to the f32 input),
            # so plain Sync HWDGE loads land f32r tiles directly and the
            # f32r transposes/mm1 see "rounded" inputs with zero extra cost.
            # ALL loads are emitted up front: stores (emitted later on the
            # same queue) can never delay load bytes.
            WAVES = [(0, 512), (512, 512), (1024, 1024), (2048, 1024), (3072, 1024)]
            qr_all = [[qrp.tile([P, HW], f32r, tag="qr", name=f"qr{b}_{i}")
                       for i in range(CB)] for b in range(NB)]
            for b in range(NB):
                for (w0, wlen) in WAVES:
                    for mi in range(CB):
                        nc.sync.dma_start(
                            out=qr_all[b][mi][:, w0:w0 + wlen],
                            in_=x_ext[b, mi * P:(mi + 1) * P, w0:w0 + wlen],
                        )

            ident_f = const.tile([P, P], f32)
            make_identity(nc, ident_f)
            ident_r = const.tile([P, P], f32r)
            nc.vector.tensor_copy(ident_r[:], ident_f[:])
            gamma_sb = const.tile([P, 1], f32)
            nc.sync.dma_start(out=gamma_sb[:], in_=g_ext[:].to_broadcast([P, 1]))

            # dummy matmuls while the first loads land: warms the PE clock
            # gate (HAM) so real matmuls start at full rate
            warm = psA.tile([P, C], f32, tag="psA", name="warmup")
            for i in range(16):
                nc.tensor.matmul(warm[:, :P], ident_f[:], ident_f[:],
                                 start=True, stop=True)

            C0S = [min(mi * P, 2 * P) for mi in range(CB)]  # 0,128,256,256

            def phase1(b, filler=None):
                """q8 cast-DMAs, transpose to qT, sim matmuls (upper-tri).

                filler: list of closures; one is emitted after every other
                transpose/mm1 group from kn>=F_START (keeping a 2-chunk
                reserve) so the PE chews the previous batch's mm2 while this
                batch streams.
                """
                st = {"qr": qr_all[b]}
                st["q8"] = q8p.tile([P, CB, HW], fp8, tag="q8", name=f"q8_{b}")
                st["psim"] = [psimp.tile([P, C], f32, tag="psim",
                                         name=f"psim{b}_{i}") for i in range(CB)]
                qr_t, psim, q8_t = st["qr"], st["psim"], st["q8"]
                qt_tiles = {}

                def mm1(kn):
                    for mi in range(CB):
                        c0 = C0S[mi]
                        nc.tensor.matmul(
                            psim[mi][:, c0:],
                            qt_tiles[kn][:, mi * P:(mi + 1) * P],
                            qt_tiles[kn][:, c0:],
                            start=(kn == 0),
                            stop=(kn == KN - 1),
                        )

                F_START = 20
                pending = []
                for (w0, wlen) in WAVES:
                    for kq in range(wlen // P):
                        kn = w0 // P + kq
                        pst = psA.tile([P, C], f32r, tag="psA")
                        for ci in range(CB):
                            nc.tensor.transpose(
                                pst[:, ci * P:(ci + 1) * P],
                                qr_t[ci][:, kn * P:(kn + 1) * P],
                                ident_r[:],
                            )
                        qt = qtp.tile([P, C], f32r, tag="qt", name=f"qt{b}_{kn}")
                        qt_tiles[kn] = qt
                        copyback(qt[:], pst[:])
                        pending.append(kn)
                        if len(pending) > 2:
                            mm1(pending.pop(0))
                        if filler and len(filler) > 4 and kn >= F_START:
                            filler.pop(0)()
                    # fp8 cast of this wave: wide CAST copies, DVE/ACT split
                    for mi in range(CB):
                        if (kn + mi) % 2 == 0:
                            nc.vector.tensor_copy(
                                q8_t[:, mi, w0:w0 + wlen],
                                qr_t[mi][:, w0:w0 + wlen],
                            )
                        else:
                            nc.scalar.copy(
                                q8_t[:, mi, w0:w0 + wlen],
                                qr_t[mi][:, w0:w0 + wlen],
                            )
                for kn in pending:
                    mm1(kn)
                return st

            def softmax_pt(b, st):
                """tri fills, rowwise softmax, p8T = fp8(T(p)), rzg = gamma/Z."""
                psim = st["psim"]
                for (i, j) in [(1, 0), (2, 0), (2, 1), (3, 0), (3, 1)]:
                    tmp = trip.tile([P, P], f32r, tag="tri")
                    nc.scalar.copy(tmp[:], psim[j][:, i * P:(i + 1) * P])
                    nc.tensor.transpose(
                        psim[i][:, j * P:(j + 1) * P].bitcast(f32r),
                        tmp[:], ident_r[:],
                    )
                p_t = []
                rzg_t = []
                for mi in range(CB):
                    mrow = vec.tile([P, 1], f32, tag="mrow")
                    nc.vector.tensor_reduce(
                        mrow[:], psim[mi][:], axis=AX.X, op=ALU.min
                    )
                    zrow = vec.tile([P, 1], f32, tag="zrow")
                    pt = pp.tile([P, C], f32r, tag="p", bufs=5)
                    nc.scalar.activation(
                        pt[:], psim[mi][:], ACTF.Exp,
                        bias=mrow[:], scale=-1.0, accum_out=zrow[:],
                    )
                    rz = vec.tile([P, 1], f32, tag="rz")
                    nc.vector.reciprocal(rz[:], zrow[:])
                    rzg = vec.tile([P, 1], f32, tag="rzg", name=f"rzg{b}_{mi}")
                    nc.vector.tensor_mul(rzg[:], rz[:], gamma_sb[:])
                    p_t.append(pt)
                    rzg_t.append(rzg)
                p8t = p8tp.tile([P, CB, C], fp8, tag="p8t", name=f"p8t_{b}")
                for kd in range(CB):
                    pst = psA.tile([P, C], f32r, tag="psA")
                    for ci in range(CB):
                        nc.tensor.transpose(
                            pst[:, ci * P:(ci + 1) * P],
                            p_t[ci][:, kd * P:(kd + 1) * P],
                            ident_r[:],
                        )
                    nc.vector.tensor_copy(p8t[:, kd, :], pst[:])
                st["p8t"] = p8t
                st["rzg"] = rzg_t

            def m2_chunks(b, st):
                """per-(mi, quarter) closures: 2 DoubleRow mm2 chunks, fused
                scale+residual (DVE) or scale+add (ACT+Pool), store on Sync."""
                qr_t, q8_t, p8t, rzg = st["qr"], st["q8"], st["p8t"], st["rzg"]
                chunks = []

                def make(mi, qtr):
                    def emit():
                        fine = (b == NB - 1 and mi == CB - 1)
                        stg = osb.tile([P, HW // 4], f32, tag="ot")
                        for njh in range(NJ // 4):
                            nj = qtr * (NJ // 4) + njh
                            pf = pfeat.tile([P, 512], f32, tag="pf")
                            for t in range(2):
                                nc.tensor.matmul(
                                    pf[:],
                                    p8t[:, 2 * t:2 * t + 2, mi * P:(mi + 1) * P],
                                    q8_t[:, 2 * t:2 * t + 2, nj * 512:(nj + 1) * 512],
                                    start=(t == 0), stop=(t == 1),
                                    perf_mode=DR,
                                )
                            dst = stg[:, njh * 512:(njh + 1) * 512]
                            src_x = qr_t[mi][:, nj * 512:(nj + 1) * 512]
                            slot = _flip[0] % 4
                            if slot < 2:
                                nc.vector.scalar_tensor_tensor(
                                    dst, pf[:], rzg[mi][:], src_x,
                                    op0=ALU.mult, op1=ALU.add,
                                )
                            else:
                                # gpsimd can't read PSUM: ACT scales out of
                                # PSUM, then DVE/Pool add the residual in SBUF
                                nc.scalar.activation(
                                    dst, pf[:], ACTF.Copy, scale=rzg[mi][:]
                                )
                                if slot == 2:
                                    nc.vector.tensor_add(dst, dst, src_x)
                                else:
                                    nc.gpsimd.tensor_add(dst, dst, src_x)
                            _flip[0] += 1
                            if fine:
                                nc.sync.dma_start(
                                    out=o_ext[b, mi * P:(mi + 1) * P,
                                              nj * 512:(nj + 1) * 512],
                                    in_=dst,
                                )
                        if not fine:
                            nc.sync.dma_start(
                                out=o_ext[b, mi * P:(mi + 1) * P,
                                          qtr * (HW // 4):(qtr + 1) * (HW // 4)],
                                in_=stg[:],
                            )
                    return emit

                for mi in range(CB):
                    for qtr in range(4):
                        chunks.append(make(mi, qtr))
                return chunks

            st0 = phase1(0)
            softmax_pt(0, st0)
            m2q0 = m2_chunks(0, st0)
            st1 = phase1(1, filler=m2q0)
            for emit in m2q0:       # reserve covers softmax(1) latency
                emit()
            softmax_pt(1, st1)
            for emit in m2_chunks(1, st1):
                emit()

    nc.finalize()
    return nc


def get_bass():
    if "nc" not in _BUILD_CACHE:
        _BUILD_CACHE["nc"] = build_bass()
    return _BUILD_CACHE["nc"]


def make_in_maps(x, gamma):
    x = np.ascontiguousarray(np.asarray(x, dtype=np.float32)).reshape(B, C, HW)
    gamma = np.asarray(gamma, dtype=np.float32).reshape(1)
    return [
        {"x": x[i * NB:(i + 1) * NB], "gamma": gamma}
        for i in range(NCORES)
    ]


def run(x, gamma, trace=False, **trace_kwargs):
    from concourse.bass_utils import run_bass_kernel_spmd

    nc = get_bass()
    res = run_bass_kernel_spmd(
        nc, make_in_maps(x, gamma), core_ids=list(range(NCORES)),
        trace=trace, **trace_kwargs,
    )
    out = np.concatenate([res.results[i]["out"] for i in range(NCORES)], axis=0)
    return out.reshape(B, C, H, W), res


def kernel(x, gamma):
    out, _ = run(x, gamma, trace=False)
    return out


# revision 19
# speedup vs baseline: 1.1783x; 1.1783x over previous
"""Channel-attention (CAM) Trainium2 kernel — fp8 DoubleRow mm2 version.

Reference computation (per batch b of 16):
    q   = x[b].reshape(C, HW)                  # C=512, HW=4096
    sim = q @ q.T                              # [C, C], symmetric
    attn = softmax(max(sim) - sim, axis=-1)    # == exp(min_r - sim) / Z_r
    out[b] = gamma * attn @ q + x[b]

Sharding: data-parallel over batch across 8 NeuronCores (2 batches/core).
kernel() takes full inputs, shards internally, returns the full output.

Per-core kernel design (v3):
  - ALL loads (both batches) are emitted up front on the Sync HWDGE queue;
    stores are emitted later on the same queue so store bytes can never
    delay load bytes. x lands directly in persistent qr f32 tiles.
  - sim (mm1) in f32r: winner-take-all softmax needs sim errors << the
    typical min-gap; f32r gives ~4e-4 end-to-end rel err. qT tiles are
    PE-transposed from qr (bf16 identity: 1 cycle/row) and rounded to
    f32r by the PSUM->SBUF copyback cast (BIR verifier requirement).
  - mm2 in fp8e4 with DoubleRow perf mode (2 k-rows/cycle, verified
    213ns per 512-col call on HW): p = exp(min-sim) is cast to fp8
    UNSCALED so the softmax winner is exactly 1.0 in fp8; q8 = fp8(q) is
    produced by gpsimd software-DGE cast-DMAs (SBUF->SBUF, zero vector
    engine cost). gamma/Z is applied post-matmul fused with the +x
    residual: chunks alternate DVE scalar_tensor_tensor (one pass) and
    ACT scale + Pool add (two ops) for engine balance. Accuracy ~1.1e-2
    rel L2, dominated by fp8 quantization of q weighted by gamma=0.435.
  - DoubleRow layout: q8 [128, 4(kd), 4096], p8T [128, 4(kd), 512(c)];
    each mm2 call contracts a kd pair: lhsT [128,2,128], rhs [128,2,512].
  - Schedule: batch-0's mm2 chunks interleave between batch-1's
    transpose/mm1 groups as PE filler, with a 2-chunk reserve to cover
    batch-1's softmax latency (HAM punishes PE gaps with ~3us of
    half-rate clock ramp).
  - 8 dummy identity matmuls at t=0 pre-warm the PE clock gate while the
    first loads are in flight.
"""
import sys

if "/opt/trn_rl_repo" not in sys.path:
    sys.path.insert(0, "/opt/trn_rl_repo")

import numpy as np

B, C, H, W = 16, 512, 64, 64
HW = H * W
NCORES = 8
NB = B // NCORES          # batches per core
P = 128
CB = C // P               # 4 channel blocks
KN = HW // P              # 32 contraction chunks for sim
NJ = HW // 512            # 8 output column chunks

_BUILD_CACHE = {}


def build_bass():
    import concourse.bacc as bacc
    import concourse.tile as tile
    from concourse import mybir
    from concourse.masks import make_identity

    f32 = mybir.dt.float32
    f32r = mybir.dt.float32r
    bf16 = mybir.dt.bfloat16
    fp8 = mybir.dt.float8e4
    AX = mybir.AxisListType
    ALU = mybir.AluOpType
    ACTF = mybir.ActivationFunctionType
    DR = mybir.MatmulPerfMode.DoubleRow

    nc = bacc.Bacc()
    x_ext = nc.declare_dram_parameter("x", [NB, C, HW], f32r, isOutput=False)
    g_ext = nc.declare_dram_parameter("gamma", [1], f32, isOutput=False)
    o_ext = nc.declare_dram_parameter("out", [NB, C, HW], f32, isOutput=True)

    _flip = [0]

    with tile.TileContext(nc) as tc:
        with (
            tc.tile_pool(name="const", bufs=1) as const,
            tc.tile_pool(name="qr", bufs=2 * CB) as qrp,
            tc.tile_pool(name="q8", bufs=2) as q8p,
            tc.tile_pool(name="qt", bufs=6) as qtp,
            tc.tile_pool(name="pp", bufs=5) as pp,
            tc.tile_pool(name="p8t", bufs=2) as p8tp,
            tc.tile_pool(name="osb", bufs=3) as osb,
            tc.tile_pool(name="tri", bufs=2) as trip,
            tc.tile_pool(name="vec", bufs=32) as vec,
            tc.tile_pool(name="psA", bufs=2, space="PSUM") as psA,
            tc.tile_pool(name="psim", bufs=4, space="PSUM") as psimp,
            tc.tile_pool(name="pfeat", bufs=2, space="PSUM") as pfeat,
        ):
            _cb = [0]

            def copyback(dst, src):
                # qt copybacks mostly on ACT; DVE owns the mm2 out-units
                if _cb[0] % 4 == 3:
                    nc.vector.tensor_copy(dst, src)
                else:
                    nc.scalar.copy(dst, src)
                _cb[0] += 1

            # x is declared f32r in DRAM (byte-identical to the f32 input),
            # so plain Sync HWDGE loads land f32r tiles directly and the
            # f32r transposes/mm1 see "rounded" inputs with zero extra cost.
            # ALL loads are emitted up front: stores (emitted later on the
            # same queue) can never delay load bytes.
            WAVES = [(0, 512), (512, 512), (1024, 1024), (2048, 1024), (3072, 1024)]
            qr_all = [[qrp.tile([P, HW], f32r, tag="qr", name=f"qr{b}_{i}")
                       for i in range(CB)] for b in range(NB)]
            for b in range(NB):
                for (w0, wlen) in WAVES:
                    for mi in range(CB):
                        nc.sync.dma_start(
                            out=qr_all[b][mi][:, w0:w0 + wlen],
                            in_=x_ext[b, mi * P:(mi + 1) * P, w0:w0 + wlen],
                        )

            ident_f = const.tile([P, P], f32)
            make_identity(nc, ident_f)
            ident_r = const.tile([P, P], f32r)
            nc.vector.tensor_copy(ident_r[:], ident_f[:])
            gamma_sb = const.tile([P, 1], f32)
            nc.sync.dma_start(out=gamma_sb[:], in_=g_ext[:].to_broadcast([P, 1]))

            # dummy matmuls while the first loads land: warms the PE clock
            # gate (HAM) so real matmuls start at full rate
            warm = psA.tile([P, C], f32, tag="psA", name="warmup")
            for i in range(16):
                nc.tensor.matmul(warm[:, :P], ident_f[:], ident_f[:],
                                 start=True, stop=True)

            C0S = [min(mi * P, 2 * P) for mi in range(CB)]  # 0,128,256,256

            def phase1(b, filler=None):
                """q8 cast-DMAs, transpose to qT, sim matmuls (upper-tri).

                filler: list of closures; one is emitted after every other
                transpose/mm1 group from kn>=F_START (keeping a 2-chunk
                reserve) so the PE chews the previous batch's mm2 while this
                batch streams.
                """
                st = {"qr": qr_all[b]}
                st["q8"] = q8p.tile([P, CB, HW], fp8, tag="q8", name=f"q8_{b}")
                st["psim"] = [psimp.tile([P, C], f32, tag="psim",
                                         name=f"psim{b}_{i}") for i in range(CB)]
                qr_t, psim, q8_t = st["qr"], st["psim"], st["q8"]
                qt_tiles = {}

                def mm1(kn):
                    for mi in range(CB):
                        c0 = C0S[mi]
                        nc.tensor.matmul(
                            psim[mi][:, c0:],
                            qt_tiles[kn][:, mi * P:(mi + 1) * P],
                            qt_tiles[kn][:, c0:],
                            start=(kn == 0),
                            stop=(kn == KN - 1),
                        )

                F_START = 8
                pending = []
                for (w0, wlen) in WAVES:
                    for kq in range(wlen // P):
                        kn = w0 // P + kq
                        pst = psA.tile([P, C], f32r, tag="psA")
                        for ci in range(CB):
                            nc.tensor.transpose(
                                pst[:, ci * P:(ci + 1) * P],
                                qr_t[ci][:, kn * P:(kn + 1) * P],
                                ident_r[:],
                            )
                        qt = qtp.tile([P, C], f32r, tag="qt", name=f"qt{b}_{kn}")
                        qt_tiles[kn] = qt
                        copyback(qt[:], pst[:])
                        pending.append(kn)
                        if len(pending) > 2:
                            mm1(pending.pop(0))
                        if (filler and len(filler) > 4 and kn >= F_START
                                and kn % 2 == 0):
                            filler.pop(0)()
                    # fp8 cast of this wave: wide CAST copies, DVE/ACT split
                    for mi in range(CB):
                        if (kn + mi) % 2 == 0:
                            nc.vector.tensor_copy(
                                q8_t[:, mi, w0:w0 + wlen],
                                qr_t[mi][:, w0:w0 + wlen],
                            )
                        else:
                            nc.scalar.copy(
                                q8_t[:, mi, w0:w0 + wlen],
                                qr_t[mi][:, w0:w0 + wlen],
                            )
                for kn in pending:
                    mm1(kn)
                return st

            def softmax_pt(b, st):
                """tri fills, rowwise softmax, p8T = fp8(T(p)), rzg = gamma/Z."""
                psim = st["psim"]
                for (i, j) in [(1, 0), (2, 0), (2, 1), (3, 0), (3, 1)]:
                    tmp = trip.tile([P, P], f32r, tag="tri")
                    nc.scalar.copy(tmp[:], psim[j][:, i * P:(i + 1) * P])
                    nc.tensor.transpose(
                        psim[i][:, j * P:(j + 1) * P].bitcast(f32r),
                        tmp[:], ident_r[:],
                    )
                p_t = []
                rzg_t = []
                for mi in range(CB):
                    mrow = vec.tile([P, 1], f32, tag="mrow")
                    nc.vector.tensor_reduce(
                        mrow[:], psim[mi][:], axis=AX.X, op=ALU.min
                    )
                    zrow = vec.tile([P, 1], f32, tag="zrow")
                    pt = pp.tile([P, C], f32r, tag="p", bufs=5)
                    nc.scalar.activation(
                        pt[:], psim[mi][:], ACTF.Exp,
                        bias=mrow[:], scale=-1.0, accum_out=zrow[:],
                    )
                    rz = vec.tile([P, 1], f32, tag="rz")
                    nc.vector.reciprocal(rz[:], zrow[:])
                    rzg = vec.tile([P, 1], f32, tag="rzg", name=f"rzg{b}_{mi}")
                    nc.vector.tensor_mul(rzg[:], rz[:], gamma_sb[:])
                    p_t.append(pt)
                    rzg_t.append(rzg)
                p8t = p8tp.tile([P, CB, C], fp8, tag="p8t", name=f"p8t_{b}")
                for kd in range(CB):
                    pst = psA.tile([P, C], f32r, tag="psA")
                    for ci in range(CB):
                        nc.tensor.transpose(
                            pst[:, ci * P:(ci + 1) * P],
                            p_t[ci][:, kd * P:(kd + 1) * P],
                            ident_r[:],
                        )
                    nc.vector.tensor_copy(p8t[:, kd, :], pst[:])
                st["p8t"] = p8t
                st["rzg"] = rzg_t

            def m2_chunks(b, st):
                """per-(mi, quarter) closures: 2 DoubleRow mm2 chunks, fused
                scale+residual (DVE) or scale+add (ACT+Pool), store on Sync."""
                qr_t, q8_t, p8t, rzg = st["qr"], st["q8"], st["p8t"], st["rzg"]
                chunks = []

                def make(mi, qtr):
                    def emit():
                        fine = (b == NB - 1 and mi == CB - 1)
                        stg = osb.tile([P, HW // 4], f32, tag="ot")
                        for njh in range(NJ // 4):
                            nj = qtr * (NJ // 4) + njh
                            pf = pfeat.tile([P, 512], f32, tag="pf")
                            for t in range(2):
                                nc.tensor.matmul(
                                    pf[:],
                                    p8t[:, 2 * t:2 * t + 2, mi * P:(mi + 1) * P],
                                    q8_t[:, 2 * t:2 * t + 2, nj * 512:(nj + 1) * 512],
                                    start=(t == 0), stop=(t == 1),
                                    perf_mode=DR,
                                )
                            dst = stg[:, njh * 512:(njh + 1) * 512]
                            src_x = qr_t[mi][:, nj * 512:(nj + 1) * 512]
                            slot = _flip[0] % 4
                            if slot < 2:
                                nc.vector.scalar_tensor_tensor(
                                    dst, pf[:], rzg[mi][:], src_x,
                                    op0=ALU.mult, op1=ALU.add,
                                )
                            else:
                                # gpsimd can't read PSUM: ACT scales out of
                                # PSUM, then DVE/Pool add the residual in SBUF
                                nc.scalar.activation(
                                    dst, pf[:], ACTF.Copy, scale=rzg[mi][:]
                                )
                                if slot == 2:
                                    nc.vector.tensor_add(dst, dst, src_x)
                                else:
                                    nc.gpsimd.tensor_add(dst, dst, src_x)
                            _flip[0] += 1
                            if fine:
                                nc.sync.dma_start(
                                    out=o_ext[b, mi * P:(mi + 1) * P,
                                              nj * 512:(nj + 1) * 512],
                                    in_=dst,
                                )
                        if not fine:
                            nc.sync.dma_start(
                                out=o_ext[b, mi * P:(mi + 1) * P,
                                          qtr * (HW // 4):(qtr + 1) * (HW // 4)],
                                in_=stg[:],
                            )
                    return emit

                for mi in range(CB):
                    for qtr in range(4):
                        chunks.append(make(mi, qtr))
                return chunks

            st0 = phase1(0)
            softmax_pt(0, st0)
            m2q0 = m2_chunks(0, st0)
            st1 = phase1(1, filler=m2q0)
            for emit in m2q0:       # reserve covers softmax(1) latency
                emit()
            softmax_pt(1, st1)
            for emit in m2_chunks(1, st1):
                emit()

    nc.finalize()
    return nc


def get_bass():
    if "nc" not in _BUILD_CACHE:
        _BUILD_CACHE["nc"] = build_bass()
    return _BUILD_CACHE["nc"]


def make_in_maps(x, gamma):
    x = np.ascontiguousarray(np.asarray(x, dtype=np.float32)).reshape(B, C, HW)
    gamma = np.asarray(gamma, dtype=np.float32).reshape(1)
    return [
        {"x": x[i * NB:(i + 1) * NB], "gamma": gamma}
        for i in range(NCORES)
    ]


def run(x, gamma, trace=False, **trace_kwargs):
    from concourse.bass_utils import run_bass_kernel_spmd

    nc = get_bass()
    res = run_bass_kernel_spmd(
        nc, make_in_maps(x, gamma), core_ids=list(range(NCORES)),
        trace=trace, **trace_kwargs,
    )
    out = np.concatenate([res.results[i]["out"] for i in range(NCORES)], axis=0)
    return out.reshape(B, C, H, W), res


def kernel(x, gamma):
    out, _ = run(x, gamma, trace=False)
    return out
